# revision 55
# baseline (speedup 1.0000x reference)
"""Causal self-attention Bass/Tile kernel for Trainium2, SPMD over 8 NeuronCores.

Sharding (hybrid DP x TP): core c owns batch u = c//4 and head-quad
q = c%4 (heads [4q, 4q+4), a 256-wide slice of the hidden dim). Each
core computes, for its batch's m = 2048 tokens, the q/k/v projections
for its 4 heads, causal attention, and the partial output projection
out_c = h_c @ Wo[:, slice].T; the host sums 4 partials per batch.

Attention datapath: scores are computed transposed [j, i] in fp16
(fp32 PSUM), exp'd on ScalarE with a constant offset EXPC subtracted
(cancels in softmax normalization; keeps exp values inside fp8-e4m3
range). Probabilities land in fp8 for off-diagonal j-blocks and bf16
for the 128-col diagonal stripe of each query sub-block (exact early
rows + the GPSIMD triangular mask). The PV product is computed in
[i, d] layout - out[i_block(128), head, 65] - so each matmul streams
only 65 columns (64 v dims + a ones column that yields the softmax
denominator): off-diagonal j-block PAIRS via the fp8 DoubleRow perf
mode (2 k-tiles per instruction at 0.5 cycles/row), leftover/diagonal
blocks as 65-cycle singles. Both heads accumulate into ONE PSUM bank
as a single accumulation group. Normalization is a DVE reciprocal +
per-partition tensor_scalar multiply; a PE transpose (identity matmul)
restores hT[d, m] for the output projection. The v-projection for
m-tiles >= 1 also runs in fp8 DoubleRow (4x fewer PE cycles); m-tile 0
stays fp16 so rows < 512 (which attend only those keys) see exact v.
Everything else runs in fp16 (same PE speed as bf16, 8x the mantissa).

Scheduling: the PE executes its queue strictly in order, so emission
order is the schedule, and the binding resource is BOTH the PE and
ScalarE's exp stream (~83us each). Emission is organized in per-window
phases A (pair-0 scores+exp) and B (pair-1), with each pair's PV
sweeps woven into the NEXT score phase. All other work - projections,
out-proj chunks, PV units - lives in a deadline/gate-tagged work queue
and is paced into the score stream against the accumulated ScalarE
deficit (scaled per window: early windows under-feed since ScalarE is
not saturated there, the last window over-feeds), so the PE idles as
little as possible exactly where ScalarE is the bottleneck. The tail
interleaves the last PV sweeps with the final out-proj chunks, copies
on alternating engines, and one DMA per 128-row block on alternating
queues to minimize the drain.
"""

import os
import sys

sys.path.insert(0, "/opt/trn_rl_repo")
os.environ.setdefault("MYCRO_LOCAL_CACHE", "1")

from collections import deque
from contextlib import ExitStack

import numpy as np
import ml_dtypes

import concourse.bass as bass
import concourse.tile as tile
from concourse import bacc, mybir

F32 = mybir.dt.float32
BF16 = mybir.dt.bfloat16
FP16 = mybir.dt.float16
FP8 = mybir.dt.float8e4
BF = ml_dtypes.bfloat16
DoubleRow = mybir.MatmulPerfMode.DoubleRow

B, S, D = 2, 2048, 1024
H, HD = 16, 64
NCORES = 8
HS = 256          # head slice per core (4 heads)
KC = D // 128     # contraction chunks for projections
M = S             # tokens per core (one batch)
NMT = M // 512    # projection m-tiles
NIT = M // 512    # attention i-windows
NJB = M // 128    # j-blocks
EXPC = 1.75       # exp offset: p = exp(s/sqrt(hd) - EXPC); cancels in softmax

# module-level knobs for test harness
PROFILE = False
LAST_EXEC_NS = None
LAST_RESULTS = None

_PROGRAM_CACHE = {}


def _emit(tc, out, xT, xT8, wq, wk, wv, wv8, wo, tri, ident):
    nc = tc.nc
    dma = nc.default_dma_engine
    escale = 1.0 / np.sqrt(HD)

    ctx = ExitStack()
    with ctx:
        consts = ctx.enter_context(tc.tile_pool(name="consts", bufs=1))
        persist = ctx.enter_context(tc.tile_pool(name="persist", bufs=1))
        xts = ctx.enter_context(tc.tile_pool(name="xts", bufs=3))
        # fp8 probability tiles, one per j-block pair; retained for the
        # whole i-window (PV sweeps replay them), so the ring holds a full
        # window (8) plus pipelining slack
        pab2p = ctx.enter_context(tc.tile_pool(name="pab2p", bufs=22))
        pabDp = ctx.enter_context(tc.tile_pool(name="pabDp", bufs=10))
        hsbp = ctx.enter_context(tc.tile_pool(name="hsbp", bufs=6))
        osbp = ctx.enter_context(tc.tile_pool(name="osbp", bufs=2))
        rp = ctx.enter_context(tc.tile_pool(name="rp", bufs=6))
        wkp = ctx.enter_context(
            tc.tile_pool(name="wkp", bufs=2, space=bass.MemorySpace.PSUM)
        )
        sbp = ctx.enter_context(
            tc.tile_pool(name="sbp", bufs=2, space=bass.MemorySpace.PSUM)
        )
        oxp = ctx.enter_context(
            tc.tile_pool(name="oxp", bufs=2, space=bass.MemorySpace.PSUM)
        )

        wq_sb = consts.tile([128, KC, HS], FP16)
        wk_sb = consts.tile([128, KC, HS], FP16)
        wv_sb = consts.tile([128, KC, HS], FP16)
        wv8_sb = consts.tile([128, KC, HS], FP8)
        wo_sb = consts.tile([128, 2, D], FP16)
        tri_sb = consts.tile([128, 2, 128], BF16)
        id_sb = consts.tile([128, 128], FP16)
        # per-partition bias AP holding -EXPC for the exp activations
        cb = consts.tile([128, 1], F32)

        # persistent activations: [128 dims-of-pair, pair, m]
        qT = persist.tile([128, 2, M], FP16)
        kT = persist.tile([128, 2, M], FP16)
        hT = persist.tile([128, 2, M], FP16)
        # v blocks [j, d] + ones column: [j, pair, head, jb, d(65)]
        # fp8 copy feeds the DoubleRow PV; bf16 copy feeds diagonal singles
        vext8 = persist.tile([128, 2, 2, NJB, 65], FP8)
        vextD = persist.tile([128, 2, 2, NJB, 65], BF16)

        xt_tiles = {}
        xt8_tiles = {}

        def issue_x_dma(mt, split_first=False):
            t = xts.tile([128, KC, 512], FP16, tag="xt", name=f"xt{mt}")
            mcols = slice(mt * 512, (mt + 1) * 512)
            if split_first:
                dma.dma_start(out=t[:, 0:1, :], in_=xT[:, 0:1, mcols])
            else:
                dma.dma_start(out=t, in_=xT[:, :, mcols])
            xt_tiles[mt] = t
            if mt >= 1:
                # fp8 copy feeds the DoubleRow v-projection (m-tile 0 stays
                # fp16: rows < 512 attend only m-tile-0 keys, so the v they
                # see is exact; later rows average the fp8 noise away)
                t8 = xts.tile([128, KC, 512], FP8, tag="xt8", name=f"xt8_{mt}")
                dma.dma_start(out=t8, in_=xT8[:, :, mcols])
                xt8_tiles[mt] = t8

        # ---- prologue DMAs: weights on the Act hwdge queue, activations on
        # the SP queue, so the two streams transfer in parallel and the PE
        # can start as soon as the first chunks of each land ----
        dma.dma_start(out=wq_sb[:, 0:1, :], in_=wq[:, 0:1, :])
        issue_x_dma(0, split_first=True)
        dma.dma_start(out=wq_sb[:, 1:KC, :], in_=wq[:, 1:KC, :])
        dma.dma_start(out=xt_tiles[0][:, 1:4, :], in_=xT[:, 1:4, 0:512])
        dma.dma_start(out=xt_tiles[0][:, 4:KC, :], in_=xT[:, 4:KC, 0:512])
        dma.dma_start(out=wk_sb, in_=wk)
        # tri and wv are consumed by window-0 masks / v-projection (~8us);
        # their descriptor-gens must precede the big x1 transfers
        dma.dma_start(out=tri_sb, in_=tri)
        dma.dma_start(out=wv_sb, in_=wv)
        issue_x_dma(1)
        dma.dma_start(out=wv8_sb, in_=wv8)
        dma.dma_start(out=id_sb, in_=ident)
        dma.dma_start(out=wo_sb, in_=wo)
        # ones columns for the softmax denominator
        nc.vector.memset(vext8[:, :, :, :, 64], 1.0)
        nc.vector.memset(vextD[:, :, :, :, 64], 1.0)
        nc.vector.memset(cb, -EXPC)

        # ---------------- work items (PE filler units) ----------------

        def proj_qk_item(mt, p, w_sb, dstT):
            def emit():
                xt = xt_tiles[mt]
                acc = wkp.tile([128, 512], F32, tag="wk", name="acc")
                for kc in range(KC):
                    nc.tensor.matmul(
                        acc,
                        lhsT=w_sb[:, kc, p * 128 : (p + 1) * 128],
                        rhs=xt[:, kc, :],
                        start=(kc == 0),
                        stop=(kc == KC - 1),
                    )
                nc.vector.tensor_copy(
                    dstT[:, p, mt * 512 : (mt + 1) * 512], acc
                )
            return emit

        def proj_v_item(mt, half):
            def emit():
                vac = wkp.tile([128, 2, 2, 2, 64], F32, tag="wk", name="vac")
                for dlt in range(2):
                    jj = half * 2 + dlt
                    jcols = slice(jj * 128, (jj + 1) * 128)
                    if mt >= 1:
                        # fp8 DoubleRow: two 128-chunks contracted per
                        # instruction at half rate -> 4x fewer PE cycles
                        xt8 = xt8_tiles[mt]
                        for kp in range(KC // 2):
                            nc.tensor.matmul(
                                vac[:, dlt],
                                lhsT=xt8[:, 2 * kp : 2 * kp + 2, jcols],
                                rhs=wv8_sb[:, 2 * kp : 2 * kp + 2, :],
                                start=(kp == 0),
                                stop=(kp == KC // 2 - 1),
                                perf_mode=DoubleRow,
                            )
                    else:
                        xt = xt_tiles[mt]
                        for kc in range(KC):
                            nc.tensor.matmul(
                                vac[:, dlt],
                                lhsT=xt[:, kc, jcols],
                                rhs=wv_sb[:, kc, :],
                                start=(kc == 0),
                                stop=(kc == KC - 1),
                            )
                for dlt in range(2):
                    jbg = mt * 4 + half * 2 + dlt
                    nc.vector.tensor_copy(vext8[:, :, :, jbg, 0:64], vac[:, dlt])
                    nc.vector.tensor_copy(vextD[:, :, :, jbg, 0:64], vac[:, dlt])
            return emit

        def proj_qk_items(mt):
            return [
                proj_qk_item(mt, 0, wq_sb, qT),
                proj_qk_item(mt, 0, wk_sb, kT),
                proj_qk_item(mt, 1, wq_sb, qT),
                proj_qk_item(mt, 1, wk_sb, kT),
            ]

        def proj_items(mt):
            return proj_qk_items(mt) + [proj_v_item(mt, 0), proj_v_item(mt, 1)]

        osb_tiles = {}

        def s3_item(iwin, q4, ch, fine_dma=False, use_act=False):
            def emit():
                if iwin not in osb_tiles:
                    osb_tiles[iwin] = osbp.tile(
                        [128, 4, D], FP16, tag="osb", name=f"osb{iwin}"
                    )
                osb = osb_tiles[iwin]
                blk = slice(iwin * 512 + q4 * 128, iwin * 512 + (q4 + 1) * 128)
                cs = slice(ch * 512, (ch + 1) * 512)
                op = wkp.tile([128, 512], F32, tag="wk", name="op")
                for p in range(2):
                    nc.tensor.matmul(
                        op,
                        lhsT=hT[:, p, blk],
                        rhs=wo_sb[:, p, cs],
                        start=(p == 0),
                        stop=(p == 1),
                    )
                if use_act:
                    # ScalarE is idle in the tail; use it so the PSUM bank
                    # frees without waiting on the DVE queue
                    nc.scalar.copy(osb[:, q4, cs], op)
                else:
                    nc.vector.tensor_copy(osb[:, q4, cs], op)
                # flush rows as soon as they are complete
                if fine_dma:
                    if q4 == 3:
                        # the very last row: DMA each 512-col half as soon
                        # as it lands so the final transfer is half-size
                        eng = nc.scalar if ch else dma
                        eng.dma_start(out=out[iwin, :, q4, cs], in_=osb[:, q4, cs])
                    elif ch == 1:
                        eng = nc.scalar if q4 in (1, 2) else dma
                        eng.dma_start(out=out[iwin, :, q4, :], in_=osb[:, q4, :])
                elif ch == 1 and q4 in (1, 3):
                    dma.dma_start(
                        out=out[iwin, :, q4 - 1 : q4 + 1, :],
                        in_=osb[:, q4 - 1 : q4 + 1, :],
                    )
            return emit

        def s3_fine(iwin, q4, ch2):
            """Quarter-width out-proj chunk for the drain tail: 256 cols,
            Act copy, alternating DMA queues."""
            def emit():
                osb = osb_tiles[iwin]
                blk = slice(iwin * 512 + q4 * 128, iwin * 512 + (q4 + 1) * 128)
                cs = slice(ch2 * 256, (ch2 + 1) * 256)
                op = wkp.tile([128, 256], F32, tag="wk", name="opf")
                for p in range(2):
                    nc.tensor.matmul(
                        op,
                        lhsT=hT[:, p, blk],
                        rhs=wo_sb[:, p, cs],
                        start=(p == 0),
                        stop=(p == 1),
                    )
                nc.scalar.copy(osb[:, q4, cs], op)
                eng = nc.scalar if ch2 % 2 else dma
                eng.dma_start(out=out[iwin, :, q4, cs], in_=osb[:, q4, cs])
            return emit

        def s3_items(iwin, fine_dma=False):
            return [
                s3_item(iwin, q4, ch, fine_dma)
                for q4 in range(4)
                for ch in range(2)
            ]

        # ---------------- attention ----------------
        # Per-pair phase state: scores/exp of (pair, window) fill pab tiles
        # that the PV sweeps of the same (pair, window) later replay.
        pstate = {}

        PE_NS = 1.0 / 2.4
        DEF_SCALE = [0.7, 0.8, 1.0, 1.2]
        ACT_NS = 1.0 / 1.2
        QK_COST = KC * 512 * PE_NS
        V_COST = KC * 512 * PE_NS
        S3_COST = 2 * 512 * PE_NS

        def exp_ns(it, jb):
            """ScalarE time to exp j-block jb of window it (both heads).
            Off-diagonal blocks are exp'd in PAIRS (charged at the odd jb);
            diagonal-group blocks get one bf16 exp each."""
            off = max(0, jb * 128 - it * 512)
            nexp = 1 if jb < 4 * it else (2 if jb - 4 * it < 3 else 1)
            return 2 * (512 - off) * ACT_NS + nexp * 185.0

        def score_ns(it, jb):
            off = max(0, jb * 128 - it * 512)
            return 2 * (512 - off) * PE_NS

        def scores_jb(p, it, jb, st):
            icol = it * 512
            moff = jb * 128 - icol
            off = max(0, moff)
            sab = sbp.tile([128, 2, 512], F32, tag="sab", name="sab")
            for h in range(2):
                hr = slice(h * 64, (h + 1) * 64)
                nc.tensor.matmul(
                    sab[:, h, off:512],
                    lhsT=kT[hr, p, jb * 128 : (jb + 1) * 128],
                    rhs=qT[hr, p, icol + off : icol + 512],
                    start=True,
                    stop=True,
                )
            if moff < 0:
                # full off-diagonal block: all 512 columns to fp8
                T = jb // 2
                if T not in st["pab2"]:
                    st["pab2"][T] = pab2p.tile(
                        [128, 2, 2, 512], FP8, tag="pab2", name=f"pab2_{T}"
                    )
                nc.scalar.activation(
                    st["pab2"][T][:, jb % 2, :, :],
                    sab[:, :, :],
                    mybir.ActivationFunctionType.Exp,
                    scale=escale,
                    bias=cb[:, 0:1],
                )
            else:
                # diagonal-group block (jb = 4it + q0): ONE bf16 exp of the
                # whole computed range; GPSIMD masks the 128-col diagonal
                # stripe and converts the rest to the fp8 pair tile
                q0 = moff // 128
                pd = pabDp.tile([128, 2, 128], BF16, tag="pabD", name="pd")
                st["pabD"][jb] = pd
                nc.scalar.activation(
                    pd,
                    sab[:, :, moff : moff + 128],
                    mybir.ActivationFunctionType.Exp,
                    scale=escale,
                    bias=cb[:, 0:1],
                )
                nc.gpsimd.tensor_mul(pd, pd, tri_sb)
                if q0 < 3:
                    T = jb // 2
                    if T not in st["pab2"]:
                        st["pab2"][T] = pab2p.tile(
                            [128, 2, 2, 512], FP8, tag="pab2", name=f"pab2_{T}"
                        )
                    nc.scalar.activation(
                        st["pab2"][T][:, jb % 2, :, moff + 128 : 512],
                        sab[:, :, moff + 128 : 512],
                        mybir.ActivationFunctionType.Exp,
                        scale=escale,
                        bias=cb[:, 0:1],
                    )

        def sweep_unit(p, it, q, st):
            """PV for query sub-block q: both heads accumulate into ONE
            [128, 2, 65] PSUM bank as a single accumulation group (fp8
            DoubleRow pairs + fp8 single + bf16 diagonal), then normalize
            via DVE reciprocal + per-partition scalar muls."""
            jdiag = 4 * it + q
            # padded to a full 2KB PSUM bank so each accumulation group
            # owns its zero region exclusively
            oX = oxp.tile(
                [128, 2, 65], F32, tag="ox", name="oX",
                padded_shape=[128, 2, 256],
            )
            mms = []
            npairs = jdiag // 2          # full fp8 pairs: 2T+1 <= jdiag-1
            for h in range(2):
                for T in range(npairs):
                    mms.append(("pair", h, T))
                if jdiag % 2 == 1:       # leftover fp8 single jb=jdiag-1
                    mms.append(("single", h, jdiag - 1))
                mms.append(("diag", h, jdiag))
            n = len(mms)
            qs = slice(q * 128, (q + 1) * 128)
            for idx, (kind, h, a) in enumerate(mms):
                st_, sp_ = idx == 0, idx == n - 1
                if kind == "pair":
                    nc.tensor.matmul(
                        oX[:, h, :],
                        lhsT=st["pab2"][a][:, :, h, qs],
                        rhs=vext8[:, p, h, 2 * a : 2 * a + 2, :],
                        start=st_,
                        stop=sp_,
                        perf_mode=DoubleRow,
                    )
                elif kind == "single":
                    nc.tensor.matmul(
                        oX[:, h, :],
                        lhsT=st["pab2"][a // 2][:, a % 2, h, qs],
                        rhs=vext8[:, p, h, a, :],
                        start=st_,
                        stop=sp_,
                    )
                else:  # diag, bf16
                    nc.tensor.matmul(
                        oX[:, h, :],
                        lhsT=st["pabD"][a][:, h, :],
                        rhs=vextD[:, p, h, a, :],
                        start=st_,
                        stop=sp_,
                    )
            # normalize: h_sb[i, (h,d)] = oX[i, h, 0:64] / oX[i, h, 64]
            rec = rp.tile([128, 2], F32, tag="rc", name="rec")
            nc.vector.reciprocal_approx_fast(out=rec, in_=oX[:, :, 64])
            hsb = hsbp.tile([128, 2, 64], FP16, tag="hsb", name="hsb")
            st["hsb"][q] = hsb
            for h in range(2):
                nc.vector.tensor_scalar_mul(
                    hsb[:, h, :], oX[:, h, 0:64], rec[:, h : h + 1]
                )

        def tp_unit(p, it, q, st, use_act=False):
            """PE transpose of the normalized [i, 128] block back to [d, i]."""
            tpp = oxp.tile(
                [128, 128], FP16, tag="ox", name="tpp",
                padded_shape=[128, 1024],
            )
            nc.tensor.transpose(tpp, st["hsb"][q][:, :, :], id_sb)
            icol = it * 512
            dst = hT[:, p, icol + q * 128 : icol + (q + 1) * 128]
            if use_act:
                nc.scalar.copy(dst, tpp)
            else:
                nc.vector.tensor_copy(dst, tpp)

        def pv_units(p, it):
            """Ordered PV emission units: transposes lag their sweep so the
            DVE normalize chain overlaps PE work."""
            st = pstate[(p, it)]
            units = []
            order = [("s", 0), ("s", 1), ("t", 0), ("s", 2),
                     ("t", 1), ("s", 3), ("t", 2), ("t", 3)]
            for kind, q in order:
                if kind == "s":
                    jdiag = 4 * it + q
                    cyc = 2 * ((jdiag // 2) * 32.5 + (jdiag % 2) * 65 + 65)
                    units.append(
                        (cyc * PE_NS,
                         (lambda qq: lambda: sweep_unit(p, it, qq, st))(q))
                    )
                else:
                    units.append(
                        (128 * PE_NS,
                         (lambda qq: lambda: tp_unit(p, it, qq, st))(q))
                    )
            return units

        # Global deadline-gated work queue: each entry is
        # (cost_ns, fn, gate, due) where gate/due are phase indices
        # (A(W) = 2W, B(W) = 2W+1, tail = 2*NIT).  Items may not emit
        # before their gate phase and must emit by their due phase START.
        # Dues are monotone in append order, so forcing pops from the front.
        mand = []
        spill = deque()

        def force_due(cur, hard_only=False):
            # hard entries are true pre-dependencies of the next phase's
            # scores; soft entries (PV units) may slip past the first score
            # blocks so ScalarE gets exp work before the PE burst
            i = 0
            while i < len(mand):
                cost, fn, gate, due, hard = mand[i]
                if due <= cur and (hard or not hard_only):
                    fn()
                    mand.pop(i)
                elif due <= cur:
                    i += 1
                else:
                    break

        def attn_scores(p, it, cur):
            """Emit the score+exp phase for (pair, window), weaving work
            items paced against the accumulated ScalarE-exp deficit so the
            PE stays busy exactly as long as the exps take."""
            st = {"pab2": {}, "pabD": {}, "hsb": {}}
            pstate[(p, it)] = st
            deficit = 0.0
            jb_hi = 4 * it + 4
            for jb in range(jb_hi):
                scores_jb(p, it, jb, st)
                if jb >= 2:
                    # drip the due soft items (PV units) out a couple per
                    # block instead of bursting them at the phase start
                    n = 0
                    while n < 2 and mand and mand[0][3] <= cur:
                        mand.pop(0)[1]()
                        n += 1
                    if jb == jb_hi - 1:
                        force_due(cur)
                # early windows: feed less filler (Act is not saturated
                # there, so PE idle is free and filler is saved for later);
                # the last window is over-fed to cover its exp-heavy chain
                sc = DEF_SCALE[it]
                if it == NIT - 1 and p == 1:
                    sc = 1.35
                deficit += (exp_ns(it, jb) - score_ns(it, jb)) * sc
                # at most 2 items per block, at most 1 from spill:
                # back-to-back small items stall on PSUM-ring/DVE latency
                maxpop = 3 if it >= 2 else 2
                maxspill = 2 if it == 3 else 1
                pops = spilled = 0
                while deficit > 0 and pops < maxpop:
                    hit = None
                    for i in range(min(4, len(mand))):
                        if mand[i][2] <= cur:
                            hit = i
                            break
                    if hit is not None:
                        cost, fn = mand.pop(hit)[:2]
                    elif spill and spilled < maxspill and spill[0][2] <= cur:
                        cost, fn, _ = spill.popleft()
                        spilled += 1
                    else:
                        break
                    fn()
                    deficit -= cost
                    pops += 1

        # ---------------- software-pipelined main loop ----------------
        # Phases: A(W) = pair-0 scores, B(W) = pair-1 scores, then the tail
        # (pair-1 PV of the last window + final out-proj chunks).  PV sweeps
        # of (p, W) weave into the NEXT score phase, and all projection /
        # out-proj work is deficit-paced into whichever phase ScalarE's exp
        # stream leaves the PE idle.
        proj_qk_item(0, 0, wq_sb, qT)()
        proj_qk_item(0, 0, wk_sb, kT)()
        TAIL = 2 * NIT
        for W in range(NIT):
            pa, pb, pa1, pb1 = 2 * W, 2 * W + 1, 2 * W + 2, 2 * W + 3
            if W == 0:
                mand.append((QK_COST, proj_qk_item(0, 1, wk_sb, kT), 0, pb, 1))
                mand.append((QK_COST, proj_qk_item(0, 1, wq_sb, qT), 0, pb, 1))
                mand.append((V_COST, proj_v_item(0, 0), 0, pb, 0))
                mand.append((V_COST, proj_v_item(0, 1), 0, pb, 0))
            if W + 2 < NMT:
                issue_x_dma(W + 2)
            force_due(pa, hard_only=True)
            attn_scores(0, W, pa)
            # pair-0's PV units: emitted during B(W) (or A(W+1) at latest)
            for c, f in pv_units(0, W):
                mand.append((c, f, pb, min(pa1, TAIL), 0))
            if W + 1 < NMT:
                mand.append((QK_COST, proj_qk_item(W + 1, 0, wq_sb, qT), 0, pa1, 1))
                mand.append((QK_COST, proj_qk_item(W + 1, 0, wk_sb, kT), 0, pa1, 1))
                mand.append((QK_COST, proj_qk_item(W + 1, 1, wq_sb, qT), 0, pb1, 1))
                mand.append((QK_COST, proj_qk_item(W + 1, 1, wk_sb, kT), 0, pb1, 1))
            force_due(pb, hard_only=True)
            if W >= 1:
                # reserve out-proj chunks for the exp-heaviest late phases
                s3_gate = {1: 5, 2: 6, 3: 7}[W]
                spill.extend((S3_COST, f, s3_gate) for f in s3_items(W - 1))
            attn_scores(1, W, pb)
            if W < NIT - 1:
                # pair-1's PV units weave into the next window's phases
                for c, f in pv_units(1, W):
                    mand.append((c, f, pa1, pb1, 0))
                mand.append((V_COST, proj_v_item(W + 1, 0), 0, pb1, 0))
                mand.append((V_COST, proj_v_item(W + 1, 1), 0, pb1, 0))
        # ---- tail: last window's pair-1 PV + final out-proj chunks ----
        force_due(TAIL)
        # drain most remaining out-proj chunks first so their DMAs are off
        # the critical path; keep two as weave for the normalize latency
        while len(spill) > 2:
            spill.popleft()[1]()
        u = pv_units(1, NIT - 1)
        Wl = NIT - 1
        st_l = pstate[(1, Wl)]

        def sp(n):
            for _ in range(n):
                if spill:
                    spill.popleft()[1]()



        u[0][1]()           # s0
        u[1][1]()           # s1
        sp(1)
        u[3][1]()           # s2
        tp_unit(1, Wl, 0, st_l, use_act=True)
        sp(1)
        u[5][1]()           # s3
        s3_item(Wl, 0, 0, True, False)()
        tp_unit(1, Wl, 1, st_l, use_act=True)
        s3_item(Wl, 0, 1, True, True)()
        s3_item(Wl, 1, 0, True, False)()
        tp_unit(1, Wl, 2, st_l, use_act=True)
        s3_item(Wl, 1, 1, True, True)()
        s3_item(Wl, 2, 0, True, False)()
        tp_unit(1, Wl, 3, st_l, use_act=True)
        s3_item(Wl, 2, 1, True, True)()
        s3_item(Wl, 3, 0, True, False)()
        s3_item(Wl, 3, 1, True, True)()
        while spill:
            spill.popleft()[1]()


def _declare_io(nc):
    xT = nc.dram_tensor("xT", [128, KC, M], FP16, kind="ExternalInput").ap()
    xT8 = nc.dram_tensor("xT8", [128, KC, M], FP8, kind="ExternalInput").ap()
    wv8 = nc.dram_tensor("wv8", [128, KC, HS], FP8, kind="ExternalInput").ap()
    wq = nc.dram_tensor("wq", [128, KC, HS], FP16, kind="ExternalInput").ap()
    wk = nc.dram_tensor("wk", [128, KC, HS], FP16, kind="ExternalInput").ap()
    wv = nc.dram_tensor("wv", [128, KC, HS], FP16, kind="ExternalInput").ap()
    wo = nc.dram_tensor("wo", [128, 2, D], FP16, kind="ExternalInput").ap()
    # out layout [iwin, r, q4, c]: row = iwin*512 + q4*128 + r
    out = nc.dram_tensor("out", [NIT, 128, 4, D], FP16, kind="ExternalOutput").ap()
    tri_np = np.triu(np.ones((128, 128), dtype=np.float32)).astype(BF)
    tri = nc.inline_tensor(np.ascontiguousarray(
        np.stack([tri_np, tri_np], axis=1)), "tri").ap()
    ident = nc.inline_tensor(
        np.eye(128, dtype=np.float16), "ident"
    ).ap()
    return xT, xT8, wq, wk, wv, wv8, wo, out, tri, ident


def build_program(b=B, s=S):
    key = (b, s)
    if key in _PROGRAM_CACHE:
        return _PROGRAM_CACHE[key]
    nc = bacc.Bacc("TRN2", target_bir_lowering=False, debug=False, num_devices=NCORES)
    xT, xT8, wq, wk, wv, wv8, wo, out, tri, ident = _declare_io(nc)
    with tile.TileContext(nc) as tc:
        _emit(tc, out, xT, xT8, wq, wk, wv, wv8, wo, tri, ident)
    nc.compile()
    _PROGRAM_CACHE[key] = nc
    return nc


def make_core_inputs(x, Wq, Wk, Wv, Wo):
    """Host-side sharding prep: batch u = c//4, head-quad q = c%4."""

    def wslice(W, q):
        # lhsT chunks [p, kc, j] with W[q*HS+j, kc*128+p]
        wt = W[q * HS : (q + 1) * HS, :].T.astype(np.float16)  # [D, HS]
        return np.ascontiguousarray(wt.reshape(KC, 128, HS).transpose(1, 0, 2))

    xTs = []
    xTs8 = []
    for u in range(B):
        xt = x[u].T.astype(np.float16)  # [D, M]
        xTs.append(np.ascontiguousarray(xt.reshape(KC, 128, M).transpose(1, 0, 2)))
        xTs8.append(np.ascontiguousarray(xTs[-1]).astype(ml_dtypes.float8_e4m3))

    in_maps = []
    for c in range(NCORES):
        u, q = divmod(c, 4)
        wo_t = Wo[:, q * HS : (q + 1) * HS].T.astype(np.float16)  # [HS, D]
        wv_t = Wv[q * HS : (q + 1) * HS, :].T.astype(np.float16)
        wv8 = np.ascontiguousarray(
            wv_t.reshape(KC, 128, HS).transpose(1, 0, 2)
        ).astype(ml_dtypes.float8_e4m3)
        in_maps.append(
            {
                "xT": xTs[u],
                "xT8": xTs8[u],
                "wv8": wv8,
                "wq": wslice(Wq, q),
                "wk": wslice(Wk, q),
                "wv": wslice(Wv, q),
                "wo": np.ascontiguousarray(
                    wo_t.reshape(2, 128, D).transpose(1, 0, 2)
                ),
            }
        )
    return in_maps


def kernel(x, Wq, Wk, Wv, Wo):
    global LAST_EXEC_NS, LAST_RESULTS
    x = np.asarray(x, dtype=np.float32)
    Wq = np.asarray(Wq, dtype=np.float32)
    Wk = np.asarray(Wk, dtype=np.float32)
    Wv = np.asarray(Wv, dtype=np.float32)
    Wo = np.asarray(Wo, dtype=np.float32)
    b, s, d = x.shape

    from concourse import bass_utils

    nc = build_program(b, s)
    in_maps = make_core_inputs(x, Wq, Wk, Wv, Wo)
    res = bass_utils.run_bass_kernel_spmd(
        nc, in_maps, list(range(NCORES)), trace=PROFILE
    )
    LAST_EXEC_NS = res.exec_time_ns
    LAST_RESULTS = res
    outs = []
    for u in range(B):
        acc = np.zeros((NIT, 128, 4, D), dtype=np.float64)
        for q in range(4):
            acc += np.asarray(res.results[u * 4 + q]["out"], dtype=np.float64)
        # [iwin, r, q4, c] -> [iwin, q4, r, c] -> [S, D]
        outs.append(acc.transpose(0, 2, 1, 3).reshape(S, D))
    return np.stack(outs).astype(np.float32)
